# revision 4
# baseline (speedup 1.0000x reference)
"""Trainium2 Bass kernel for nn_DriftRectifier (2-block Mamba over 64x64 images).

Sharding: data-parallel over batch B=16 -> 2 samples per core x 8 cores.
v2 architecture (engine-balanced around the DVE scan floor):
  - Vector: the 16-n selective scans (tensor_tensor_scan, DVE-only op),
    most hc multiplies, small [<=64,512] ops.
  - GpSimd: dbu multiplies (dtu*B), dtu, yo gate, some hc.
  - Scalar (ACT): exp/silu/softplus, PSUM->SBUF copies; exp/ln batched and
    the whole post phase kept inside the sqrt table set (sqrt/square/copy/
    identity coexist) to avoid ACT table thrash.
  - PE: projections, per-n y accumulation via identity matmuls, LN
    mean row-sums and [1->64] broadcasts (replaces DRAM round-trips).
  - DMA: B/C rows partition-broadcast from a DRAM staging tile; B and C
    fetched in ONE descriptor per (n, half) via a 3-dim access pattern.
Units (sample, block) are pipelined: u/zs/dtu/dt tiles are parity-
duplicated so unit k+1's projections overlap unit k's scan.
"""
import contextlib

import numpy as np

B, C, H, W = 16, 4, 64, 64
L = H * W  # 4096
DM, DI, DS, DK, DR = 64, 128, 16, 4, 4
NCORES = 8
BPC = B // NCORES  # samples per core
TC = 512           # psum / matmul chunk
NCH = L // TC      # 8 chunks
HALF = L // 2      # 2048, scan half-sequence
EPS = 1e-5

# hc (h * C) engine split: n-values in VHC run on vector, rest on gpsimd
VHC = frozenset(n for n in range(DS) if n % 3 != 0)  # 11 of 16

_CACHE = {}


def _build_program():
    import concourse.bacc as bacc
    import concourse.bass as bass
    from concourse import mybir
    from concourse.tile import TileContext

    F32 = mybir.dt.float32
    BF16 = mybir.dt.bfloat16
    AF = mybir.ActivationFunctionType
    OP = mybir.AluOpType

    nc = bacc.Bacc("TRN2")

    # ---- dram I/O ----
    zc = nc.dram_tensor("zc", [BPC, C, L], F32, kind="ExternalInput")
    out = nc.dram_tensor("out", [BPC, C, L], F32, kind="ExternalOutput")
    ident_in = nc.dram_tensor("ident", [128, 128], BF16, kind="ExternalInput")
    emb_wT = nc.dram_tensor("emb_wT", [C, DM], F32, kind="ExternalInput")
    emb_b = nc.dram_tensor("emb_b", [DM, 1], F32, kind="ExternalInput")
    head_wT = nc.dram_tensor("head_wT", [DM, C], BF16, kind="ExternalInput")
    neg_head_b = nc.dram_tensor("neg_head_b", [C, 1], F32, kind="ExternalInput")
    blk_t = []
    for m in (1, 2):
        p = f"m{m}_"
        blk_t.append({
            "cwu0": nc.dram_tensor(p + "cwu0", [2 * DM, DI], BF16, kind="ExternalInput"),
            "cwu1": nc.dram_tensor(p + "cwu1", [2 * DM, DI], BF16, kind="ExternalInput"),
            "inw_zT": nc.dram_tensor(p + "inw_zT", [DM, DI], BF16, kind="ExternalInput"),
            "conv_b": nc.dram_tensor(p + "conv_b", [DI, 1], F32, kind="ExternalInput"),
            "xpwT": nc.dram_tensor(p + "xpwT", [DI, DR + 2 * DS], BF16, kind="ExternalInput"),
            "dtpwT": nc.dram_tensor(p + "dtpwT", [DR, DI], BF16, kind="ExternalInput"),
            "dtp_b": nc.dram_tensor(p + "dtp_b", [DI, 1], F32, kind="ExternalInput"),
            "A": nc.dram_tensor(p + "A", [DI, DS], F32, kind="ExternalInput"),
            "D": nc.dram_tensor(p + "D", [DI, 1], F32, kind="ExternalInput"),
            "opwT": nc.dram_tensor(p + "opwT", [DI, DM], BF16, kind="ExternalInput"),
            "ln_g": nc.dram_tensor(p + "ln_g", [DM, 1], F32, kind="ExternalInput"),
            "ln_b": nc.dram_tensor(p + "ln_b", [DM, 1], F32, kind="ExternalInput"),
        })

    with TileContext(nc) as tc, contextlib.ExitStack() as ctx:
        consts = ctx.enter_context(tc.tile_pool(name="consts", bufs=1))
        persist = ctx.enter_context(tc.tile_pool(name="persist", bufs=1))
        bcw = ctx.enter_context(tc.tile_pool(name="bcw", bufs=2))
        enw = ctx.enter_context(tc.tile_pool(name="enw", bufs=3))
        nwork = ctx.enter_context(tc.tile_pool(name="nwork", bufs=2))
        small = ctx.enter_context(tc.tile_pool(name="small", bufs=2))
        stp = ctx.enter_context(tc.tile_pool(name="stp", bufs=1))
        psA = ctx.enter_context(tc.tile_pool(name="psA", bufs=4, space="PSUM"))
        psY = ctx.enter_context(tc.tile_pool(name="psY", bufs=1, space="PSUM"))
        dstage = ctx.enter_context(tc.tile_pool(name="dstage", bufs=2, space="DRAM"))

        # ---- constants to SBUF ----
        ident = consts.tile([128, 128], BF16)
        nc.sync.dma_start(out=ident, in_=ident_in[:])
        sb_embT = consts.tile([C, DM], F32)
        nc.sync.dma_start(out=sb_embT, in_=emb_wT[:])
        sb_embb = consts.tile([DM, 1], F32)
        nc.sync.dma_start(out=sb_embb, in_=emb_b[:])
        sb_headT = consts.tile([DM, C], BF16)
        nc.sync.dma_start(out=sb_headT, in_=head_wT[:])
        sb_nhb = consts.tile([C, 1], F32)
        nc.sync.dma_start(out=sb_nhb, in_=neg_head_b[:])
        # LN helpers: column of 1/64 (mean weights), row of ones (broadcast)
        w_mean = consts.tile([DM, 1], BF16)
        nc.vector.memset(w_mean, 1.0 / DM)
        ones1x64 = consts.tile([1, DM], F32)
        nc.vector.memset(ones1x64, 1.0)
        eps_t = consts.tile([1, 1], F32)
        nc.vector.memset(eps_t, EPS)
        one128 = consts.tile([DI, 1], F32)
        nc.vector.memset(one128, 1.0)
        blk = []
        for m in range(2):
            d = {}
            for k, t in blk_t[m].items():
                d[k] = consts.tile(list(t.shape), t.dtype, name=f"c_m{m}_{k}")
                nc.sync.dma_start(out=d[k], in_=t[:])
            blk.append(d)

        # ---- persistent tiles ----
        # feat2x: shared across units (k-post writes unit-(k+1) input after
        # k-proj has consumed it; WAR tracked by the tile framework)
        feat2x = persist.tile([2 * DM, L + 3], BF16)
        # parity-duplicated so unit k+1's proj can overlap unit k's scan/post
        u_bf = [persist.tile([DI, L], BF16, name=f"u{i}") for i in range(2)]
        zs_bf = [persist.tile([DI, L], BF16, name=f"zs{i}") for i in range(2)]
        dtu_bf = [persist.tile([DI, L], BF16, name=f"dtu{i}") for i in range(2)]
        dt_f32 = [persist.tile([DI, L], F32, name=f"dt{i}") for i in range(2)]
        yo_bf = persist.tile([DI, L], BF16)
        fch_bf = persist.tile([DM, L], BF16)
        carry = persist.tile([DI, DS], F32)

        for s in range(BPC):
            for m in range(2):
                w = blk[m]
                par = (2 * s + m) % 2
                u_t, zs_t, dtu_t, dt_t = u_bf[par], zs_bf[par], dtu_bf[par], dt_f32[par]
                bc_dram = dstage.tile([2 * DS, L], BF16, name="bc_dram")

                with nc.named_scope(f"s{s}m{m}_proj"):
                    if m == 0:
                        for c in range(NCH):
                            cs = slice(c * TC, (c + 1) * TC)
                            zch = small.tile([C, TC], F32, name="zch", tag="zch")
                            nc.sync.dma_start(out=zch, in_=zc[s][:, cs])
                            ps = psA.tile([DM, TC], F32, name="emb_ps", tag="mm")
                            nc.tensor.matmul(ps, lhsT=sb_embT, rhs=zch,
                                             start=True, stop=True)
                            nc.scalar.activation(
                                out=feat2x[0:DM, 3 + c * TC:3 + (c + 1) * TC],
                                in_=ps, func=AF.Identity, bias=sb_embb[:, :])
                            nc.scalar.activation(
                                out=feat2x[DM:2 * DM, 2 + c * TC:2 + (c + 1) * TC],
                                in_=ps, func=AF.Identity, bias=sb_embb[:, :])
                        nc.vector.memset(feat2x[0:DM, 0:3], 0.0)
                        nc.vector.memset(feat2x[DM:2 * DM, 0:2], 0.0)
                    # silu pass (conv fused into in_proj via shifted feat2x)
                    for c in range(NCH):
                        cs = slice(c * TC, (c + 1) * TC)
                        ups = psA.tile([DI, TC], F32, name="ups", tag="mm")
                        nc.tensor.matmul(ups, lhsT=w["cwu0"],
                                         rhs=feat2x[:, c * TC:c * TC + TC],
                                         start=True, stop=False)
                        nc.tensor.matmul(ups, lhsT=w["cwu1"],
                                         rhs=feat2x[:, c * TC + 2:c * TC + 2 + TC],
                                         start=False, stop=True)
                        nc.scalar.activation(out=u_t[:, cs], in_=ups, func=AF.Silu,
                                             bias=w["conv_b"][:, :])
                        zps = psA.tile([DI, TC], F32, name="zps", tag="mm")
                        nc.tensor.matmul(zps, lhsT=w["inw_zT"],
                                         rhs=feat2x[0:DM, 3 + c * TC:3 + (c + 1) * TC],
                                         start=True, stop=True)
                        nc.scalar.activation(out=zs_t[:, cs], in_=zps, func=AF.Silu)
                    # x_proj / dt pass: exps batched per half, then one ln
                    for half in range(2):
                        spe = enw.tile([DI, HALF], F32, name="spe", tag="en")
                        for cc in range(NCH // 2):
                            c = half * (NCH // 2) + cc
                            cs = slice(c * TC, (c + 1) * TC)
                            xps = psA.tile([DR + 2 * DS, TC], F32, name="xps", tag="mm")
                            nc.tensor.matmul(xps, lhsT=w["xpwT"], rhs=u_t[:, cs],
                                             start=True, stop=True)
                            # x_proj rows host-permuted to [B(16), C(16), dt(4)]
                            bcc = small.tile([2 * DS, TC], BF16, name="bcc", tag="bcc")
                            nc.scalar.activation(out=bcc, in_=xps[0:2 * DS, :],
                                                 func=AF.Copy)
                            nc.sync.dma_start(out=bc_dram[:, cs], in_=bcc)
                            dtr = small.tile([DR, TC], BF16, name="dtr", tag="dtr")
                            nc.scalar.activation(out=dtr,
                                                 in_=xps[2 * DS:2 * DS + DR, :],
                                                 func=AF.Copy)
                            dtps = psA.tile([DI, TC], F32, name="dtps", tag="mm")
                            nc.tensor.matmul(dtps, lhsT=w["dtpwT"], rhs=dtr,
                                             start=True, stop=True)
                            # softplus(x) = ln(1 + exp(x))
                            nc.scalar.activation(out=spe[:, cc * TC:(cc + 1) * TC],
                                                 in_=dtps, func=AF.Exp,
                                                 bias=w["dtp_b"][:, :])
                        nc.scalar.activation(out=dt_t[:, half * HALF:(half + 1) * HALF],
                                             in_=spe, func=AF.Ln, bias=one128[:, :])
                    nc.gpsimd.tensor_tensor(out=dtu_t, in0=dt_t, in1=u_t, op=OP.mult)

                with nc.named_scope(f"s{s}m{m}_scan"):
                    for q in range(2):
                        hs = q * HALF
                        qsl = slice(hs, hs + HALF)
                        yps = [psY.tile([DI, TC], F32, name=f"yps{k}", tag=f"yps{k}")
                               for k in range(HALF // TC)]
                        for n in range(DS):
                            en = enw.tile([DI, HALF], F32, name="en", tag="en")
                            nc.scalar.activation(out=en, in_=dt_t[:, qsl],
                                                 func=AF.Exp,
                                                 scale=w["A"][:, n:n + 1])
                            # B row n and C row n+16 in one broadcast DMA:
                            # bc[:, :HALF] = B_n, bc[:, HALF:] = C_n
                            bc_t = bcw.tile([DI, 2 * HALF], BF16, name="bc_t",
                                            tag="bc_t")
                            src = bass.AP(tensor=bc_dram.tensor,
                                          offset=bc_dram.offset + n * L + hs,
                                          ap=[[0, DI], [DS * L, 2], [1, HALF]])
                            eng = nc.sync if (n % 2 == 0) else nc.scalar
                            eng.dma_start(out=bc_t, in_=src)
                            dbu = nwork.tile([DI, HALF], BF16, name="dbu", tag="dbu")
                            nc.gpsimd.tensor_tensor(out=dbu, in0=dtu_t[:, qsl],
                                                    in1=bc_t[:, 0:HALF], op=OP.mult)
                            h_t = nwork.tile([DI, HALF], BF16, name="h_t", tag="h_t")
                            init = 0.0 if q == 0 else carry[:, n:n + 1]
                            nc.vector.tensor_tensor_scan(
                                out=h_t, data0=en, data1=dbu,
                                initial=init, op0=OP.mult, op1=OP.add)
                            if q == 0:
                                nc.vector.tensor_copy(out=carry[:, n:n + 1],
                                                      in_=h_t[:, HALF - 1:HALF])
                            hc = nwork.tile([DI, HALF], BF16, name="hc", tag="hc")
                            heng = nc.vector if n in VHC else nc.gpsimd
                            heng.tensor_tensor(out=hc, in0=h_t,
                                               in1=bc_t[:, HALF:2 * HALF], op=OP.mult)
                            for k in range(HALF // TC):
                                nc.tensor.matmul(yps[k], lhsT=ident,
                                                 rhs=hc[:, k * TC:(k + 1) * TC],
                                                 start=(n == 0), stop=(n == DS - 1))
                        for k in range(HALF // TC):
                            cs = slice(hs + k * TC, hs + (k + 1) * TC)
                            tmp = small.tile([DI, TC], F32, name="ytmp", tag="ytmp")
                            nc.vector.scalar_tensor_tensor(
                                out=tmp, in0=u_t[:, cs], scalar=w["D"][:, :],
                                in1=yps[k], op0=OP.mult, op1=OP.add)
                            nc.gpsimd.tensor_tensor(out=yo_bf[:, cs], in0=tmp,
                                                    in1=zs_t[:, cs], op=OP.mult)

                with nc.named_scope(f"s{s}m{m}_post"):
                    # out_proj + per-position layernorm over the 64 channels.
                    # Stays inside the sqrt ACT table set (sqrt/square/copy/
                    # identity) -> no table thrash.
                    for c in range(NCH):
                        cs = slice(c * TC, (c + 1) * TC)
                        fps = psA.tile([DM, TC], F32, name="fps", tag="mm")
                        nc.tensor.matmul(fps, lhsT=w["opwT"], rhs=yo_bf[:, cs],
                                         start=True, stop=True)
                        nc.scalar.activation(out=fch_bf[:, cs], in_=fps, func=AF.Copy)
                        sq = small.tile([DM, TC], BF16, name="sq", tag="sq")
                        nc.scalar.activation(out=sq, in_=fch_bf[:, cs], func=AF.Square)
                        mps = psA.tile([1, TC], F32, name="mps", tag="mm")
                        nc.tensor.matmul(mps, lhsT=w_mean, rhs=fch_bf[:, cs],
                                         start=True, stop=True)
                        mu_sb = stp.tile([1, TC], F32, name="mu_sb", tag="mu_sb")
                        nc.scalar.activation(out=mu_sb, in_=mps, func=AF.Copy)
                        qps = psA.tile([1, TC], F32, name="qps", tag="mm")
                        nc.tensor.matmul(qps, lhsT=w_mean, rhs=sq,
                                         start=True, stop=True)
                        mu2 = stp.tile([1, TC], F32, name="mu2", tag="mu2")
                        nc.vector.tensor_tensor(out=mu2, in0=mu_sb, in1=mu_sb,
                                                op=OP.mult)
                        var = stp.tile([1, TC], F32, name="var", tag="var")
                        nc.vector.tensor_tensor(out=var, in0=qps, in1=mu2,
                                                op=OP.subtract)
                        sd = stp.tile([1, TC], F32, name="sd", tag="sd")
                        nc.scalar.activation(out=sd, in_=var, func=AF.Sqrt,
                                             bias=eps_t[:, :])
                        rstd = stp.tile([1, TC], F32, name="rstd", tag="rstd")
                        nc.vector.reciprocal(out=rstd, in_=sd)
                        # broadcast mu and rstd to 64 partitions via PE
                        mbc = psA.tile([DM, TC], F32, name="mbc", tag="mm")
                        nc.tensor.matmul(mbc, lhsT=ones1x64, rhs=mu_sb,
                                         start=True, stop=True)
                        rbc = psA.tile([DM, TC], F32, name="rbc", tag="mm")
                        nc.tensor.matmul(rbc, lhsT=ones1x64, rhs=rstd,
                                         start=True, stop=True)
                        t1 = small.tile([DM, TC], BF16, name="t1", tag="t1")
                        nc.vector.tensor_tensor(out=t1, in0=fch_bf[:, cs], in1=mbc,
                                                op=OP.subtract)
                        t2 = small.tile([DM, TC], BF16, name="t2", tag="t2")
                        nc.vector.tensor_tensor(out=t2, in0=t1, in1=rbc, op=OP.mult)
                        nc.scalar.activation(
                            out=feat2x[0:DM, 3 + c * TC:3 + (c + 1) * TC],
                            in_=t2, func=AF.Identity,
                            scale=w["ln_g"][:, :], bias=w["ln_b"][:, :])
                        if m == 0:
                            nc.scalar.activation(
                                out=feat2x[DM:2 * DM, 2 + c * TC:2 + (c + 1) * TC],
                                in_=t2, func=AF.Identity,
                                scale=w["ln_g"][:, :], bias=w["ln_b"][:, :])
                        else:
                            dps = psA.tile([C, TC], F32, name="dps", tag="mm")
                            nc.tensor.matmul(
                                dps, lhsT=sb_headT,
                                rhs=feat2x[0:DM, 3 + c * TC:3 + (c + 1) * TC],
                                start=True, stop=True)
                            nd = small.tile([C, TC], F32, name="nd", tag="nd")
                            nc.scalar.activation(out=nd, in_=dps, func=AF.Identity,
                                                 scale=-1.0, bias=sb_nhb[:, :])
                            zch2 = small.tile([C, TC], F32, name="zch2", tag="zch")
                            nc.sync.dma_start(out=zch2, in_=zc[s][:, cs])
                            oc = small.tile([C, TC], F32, name="oc", tag="ytmp")
                            nc.vector.tensor_tensor(out=oc, in0=zch2, in1=nd,
                                                    op=OP.add)
                            nc.sync.dma_start(out=out[s][:, cs], in_=oc)

    nc.finalize()
    return nc


def _prep_maps(inputs):
    import ml_dtypes
    bf = ml_dtypes.bfloat16
    f = np.float32
    z = np.asarray(inputs["z_damaged"], dtype=f).reshape(B, C, L)

    base = {
        "ident": np.eye(128, dtype=bf),
        "emb_wT": np.ascontiguousarray(np.asarray(inputs["emb_w"], f).T),
        "emb_b": np.asarray(inputs["emb_b"], f).reshape(DM, 1),
        "head_wT": np.ascontiguousarray(np.asarray(inputs["head_w"], f).T).astype(bf),
        "neg_head_b": (-np.asarray(inputs["head_b"], f)).reshape(C, 1),
    }
    for m in (1, 2):
        p = f"m{m}_"
        inw = np.asarray(inputs[p + "in_proj_w"], f)  # [2DI, DM]
        w_u = inw[:DI]  # [DI, DM]
        cw = np.asarray(inputs[p + "conv_w"], f).reshape(DI, DK)
        # lhsT rows (k,m) -> cols d: w[d,k]*W_u[d,m]
        base[p + "cwu0"] = np.ascontiguousarray(np.concatenate(
            [cw[:, 0][None, :] * w_u.T, cw[:, 1][None, :] * w_u.T], axis=0)).astype(bf)
        base[p + "cwu1"] = np.ascontiguousarray(np.concatenate(
            [cw[:, 2][None, :] * w_u.T, cw[:, 3][None, :] * w_u.T], axis=0)).astype(bf)
        base[p + "inw_zT"] = np.ascontiguousarray(inw[DI:].T).astype(bf)
        base[p + "conv_b"] = np.asarray(inputs[p + "conv_b"], f).reshape(DI, 1)
        xpw = np.asarray(inputs[p + "x_proj_w"], f)  # rows: dt(4), B(16), C(16)
        xpw = np.concatenate([xpw[DR:], xpw[:DR]], axis=0)  # -> B, C, dt
        base[p + "xpwT"] = np.ascontiguousarray(xpw.T).astype(bf)
        base[p + "dtpwT"] = np.ascontiguousarray(
            np.asarray(inputs[p + "dt_proj_w"], f).T).astype(bf)
        base[p + "dtp_b"] = np.asarray(inputs[p + "dt_proj_b"], f).reshape(DI, 1)
        base[p + "A"] = -np.exp(np.asarray(inputs[p + "A_log"], f))
        base[p + "D"] = np.asarray(inputs[p + "D"], f).reshape(DI, 1)
        base[p + "opwT"] = np.ascontiguousarray(
            np.asarray(inputs[p + "out_proj_w"], f).T).astype(bf)
        base[p + "ln_g"] = np.asarray(inputs[f"ln{m}_g"], f).reshape(DM, 1)
        base[p + "ln_b"] = np.asarray(inputs[f"ln{m}_b"], f).reshape(DM, 1)

    maps = []
    for k in range(NCORES):
        mkp = dict(base)
        mkp["zc"] = np.ascontiguousarray(z[k * BPC:(k + 1) * BPC])
        maps.append(mkp)
    return maps


def _run(inputs, trace=False):
    from concourse.bass_utils import run_bass_kernel_spmd
    if "nc" not in _CACHE:
        _CACHE["nc"] = _build_program()
    nc = _CACHE["nc"]
    maps = _prep_maps(inputs)
    res = run_bass_kernel_spmd(nc, maps, core_ids=list(range(NCORES)), trace=trace)
    outs = [r["out"] for r in res.results]
    full = np.concatenate(outs, axis=0).reshape(B, C, H, W)
    return full, res


def kernel(**inputs):
    full, _ = _run(inputs, trace=False)
    return full


# revision 6
# speedup vs baseline: 1.0505x; 1.0505x over previous
"""Trainium2 Bass kernel for nn_DriftRectifier (2-block Mamba over 64x64 images).

Sharding: data-parallel over batch B=16 -> 2 samples per core x 8 cores.
v2 architecture (engine-balanced around the DVE scan floor):
  - Vector: the 16-n selective scans (tensor_tensor_scan, DVE-only op),
    most hc multiplies, small [<=64,512] ops.
  - GpSimd: dbu multiplies (dtu*B), dtu, yo gate, some hc.
  - Scalar (ACT): exp/silu/softplus, PSUM->SBUF copies; exp/ln batched and
    the whole post phase kept inside the sqrt table set (sqrt/square/copy/
    identity coexist) to avoid ACT table thrash.
  - PE: projections, per-n y accumulation via identity matmuls, LN
    mean row-sums and [1->64] broadcasts (replaces DRAM round-trips).
  - DMA: B/C rows partition-broadcast from a DRAM staging tile; B and C
    fetched in ONE descriptor per (n, half) via a 3-dim access pattern.
Units (sample, block) are pipelined: u/zs/dtu/dt tiles are parity-
duplicated so unit k+1's projections overlap unit k's scan.
"""
import contextlib

import numpy as np

B, C, H, W = 16, 4, 64, 64
L = H * W  # 4096
DM, DI, DS, DK, DR = 64, 128, 16, 4, 4
NCORES = 8
BPC = B // NCORES  # samples per core
TC = 512           # psum / matmul chunk
NCH = L // TC      # 8 chunks
HALF = L // 2      # 2048, scan half-sequence
EPS = 1e-5

# hc (h * C) engine split: n-values in VHC run on vector, rest on gpsimd
VHC = frozenset(n for n in range(DS) if n % 3 != 0)  # 11 of 16

_CACHE = {}


def _build_program():
    import concourse.bacc as bacc
    import concourse.bass as bass
    from concourse import mybir
    from concourse.tile import TileContext

    F32 = mybir.dt.float32
    BF16 = mybir.dt.bfloat16
    AF = mybir.ActivationFunctionType
    OP = mybir.AluOpType

    nc = bacc.Bacc("TRN2")

    # ---- dram I/O ----
    zc = nc.dram_tensor("zc", [BPC, C, L], F32, kind="ExternalInput")
    out = nc.dram_tensor("out", [BPC, C, L], F32, kind="ExternalOutput")
    ident_in = nc.dram_tensor("ident", [128, 128], BF16, kind="ExternalInput")
    emb_wT = nc.dram_tensor("emb_wT", [C, DM], F32, kind="ExternalInput")
    emb_b = nc.dram_tensor("emb_b", [DM, 1], F32, kind="ExternalInput")
    head_wT = nc.dram_tensor("head_wT", [DM, C], BF16, kind="ExternalInput")
    neg_head_b = nc.dram_tensor("neg_head_b", [C, 1], F32, kind="ExternalInput")
    onehot8_in = nc.dram_tensor("onehot8", [NCH, NCH * DM], F32, kind="ExternalInput")
    wsel_in = nc.dram_tensor("wsel", [DM, NCH * NCH], BF16, kind="ExternalInput")
    blk_t = []
    for m in (1, 2):
        p = f"m{m}_"
        blk_t.append({
            "cwu0": nc.dram_tensor(p + "cwu0", [2 * DM, DI], BF16, kind="ExternalInput"),
            "cwu1": nc.dram_tensor(p + "cwu1", [2 * DM, DI], BF16, kind="ExternalInput"),
            "inw_zT": nc.dram_tensor(p + "inw_zT", [DM, DI], BF16, kind="ExternalInput"),
            "conv_b": nc.dram_tensor(p + "conv_b", [DI, 1], F32, kind="ExternalInput"),
            "xpwT": nc.dram_tensor(p + "xpwT", [DI, DR + 2 * DS], BF16, kind="ExternalInput"),
            "dtpwT": nc.dram_tensor(p + "dtpwT", [DR, DI], BF16, kind="ExternalInput"),
            "dtp_b": nc.dram_tensor(p + "dtp_b", [DI, 1], F32, kind="ExternalInput"),
            "A": nc.dram_tensor(p + "A", [DI, DS], F32, kind="ExternalInput"),
            "D": nc.dram_tensor(p + "D", [DI, 1], F32, kind="ExternalInput"),
            "opwT": nc.dram_tensor(p + "opwT", [DI, DM], BF16, kind="ExternalInput"),
            "ln_g": nc.dram_tensor(p + "ln_g", [DM, 1], F32, kind="ExternalInput"),
            "ln_b": nc.dram_tensor(p + "ln_b", [DM, 1], F32, kind="ExternalInput"),
        })

    with TileContext(nc) as tc, contextlib.ExitStack() as ctx:
        consts = ctx.enter_context(tc.tile_pool(name="consts", bufs=1))
        persist = ctx.enter_context(tc.tile_pool(name="persist", bufs=1))
        bcw = ctx.enter_context(tc.tile_pool(name="bcw", bufs=2))
        enw = ctx.enter_context(tc.tile_pool(name="enw", bufs=3))
        nwork = ctx.enter_context(tc.tile_pool(name="nwork", bufs=2))
        small = ctx.enter_context(tc.tile_pool(name="small", bufs=2))
        stp = ctx.enter_context(tc.tile_pool(name="stp", bufs=1))
        psA = ctx.enter_context(tc.tile_pool(name="psA", bufs=4, space="PSUM"))
        psY = ctx.enter_context(tc.tile_pool(name="psY", bufs=1, space="PSUM"))
        dstage = ctx.enter_context(tc.tile_pool(name="dstage", bufs=2, space="DRAM"))

        # ---- constants to SBUF ----
        ident = consts.tile([128, 128], BF16)
        nc.sync.dma_start(out=ident, in_=ident_in[:])
        sb_embT = consts.tile([C, DM], F32)
        nc.sync.dma_start(out=sb_embT, in_=emb_wT[:])
        sb_embb = consts.tile([DM, 1], F32)
        nc.sync.dma_start(out=sb_embb, in_=emb_b[:])
        sb_headT = consts.tile([DM, C], BF16)
        nc.sync.dma_start(out=sb_headT, in_=head_wT[:])
        sb_nhb = consts.tile([C, 1], F32)
        nc.sync.dma_start(out=sb_nhb, in_=neg_head_b[:])
        # LN helpers: column of 1/64 (mean weights), row of ones (broadcast)
        eps8 = consts.tile([NCH, 1], F32)
        nc.vector.memset(eps8, EPS)
        oh8 = consts.tile([NCH, NCH * DM], F32)
        nc.sync.dma_start(out=oh8, in_=onehot8_in[:])
        wsel = consts.tile([DM, NCH * NCH], BF16)
        nc.sync.dma_start(out=wsel, in_=wsel_in[:])
        one128 = consts.tile([DI, 1], F32)
        nc.vector.memset(one128, 1.0)
        blk = []
        for m in range(2):
            d = {}
            for k, t in blk_t[m].items():
                d[k] = consts.tile(list(t.shape), t.dtype, name=f"c_m{m}_{k}")
                nc.sync.dma_start(out=d[k], in_=t[:])
            blk.append(d)

        # ---- persistent tiles ----
        # feat2x: shared across units (k-post writes unit-(k+1) input after
        # k-proj has consumed it; WAR tracked by the tile framework)
        feat2x = persist.tile([2 * DM, L + 3], BF16)
        # parity-duplicated so unit k+1's proj can overlap unit k's scan/post
        u_bf = [persist.tile([DI, L], BF16, name=f"u{i}") for i in range(2)]
        zs_bf = [persist.tile([DI, L], BF16, name=f"zs{i}") for i in range(2)]
        dtu_bf = [persist.tile([DI, L], BF16, name=f"dtu{i}") for i in range(2)]
        dt_f32 = [persist.tile([DI, L], F32, name=f"dt{i}") for i in range(2)]
        yo_bf = persist.tile([DI, L], BF16)
        fch_bf = persist.tile([DM, L], BF16)
        carry = persist.tile([DI, DS], F32)

        for s in range(BPC):
            for m in range(2):
                w = blk[m]
                par = (2 * s + m) % 2
                u_t, zs_t, dtu_t, dt_t = u_bf[par], zs_bf[par], dtu_bf[par], dt_f32[par]
                bc_dram = dstage.tile([2 * DS, L], BF16, name="bc_dram")

                with nc.named_scope(f"s{s}m{m}_proj"):
                    if m == 0:
                        for c in range(NCH):
                            cs = slice(c * TC, (c + 1) * TC)
                            zch = small.tile([C, TC], F32, name="zch", tag="zch")
                            nc.sync.dma_start(out=zch, in_=zc[s][:, cs])
                            ps = psA.tile([DM, TC], F32, name="emb_ps", tag="mm")
                            nc.tensor.matmul(ps, lhsT=sb_embT, rhs=zch,
                                             start=True, stop=True)
                            nc.scalar.activation(
                                out=feat2x[0:DM, 3 + c * TC:3 + (c + 1) * TC],
                                in_=ps, func=AF.Identity, bias=sb_embb[:, :])
                            nc.scalar.activation(
                                out=feat2x[DM:2 * DM, 2 + c * TC:2 + (c + 1) * TC],
                                in_=ps, func=AF.Identity, bias=sb_embb[:, :])
                        nc.vector.memset(feat2x[0:DM, 0:3], 0.0)
                        nc.vector.memset(feat2x[DM:2 * DM, 0:2], 0.0)
                    # silu pass (conv fused into in_proj via shifted feat2x)
                    for c in range(NCH):
                        cs = slice(c * TC, (c + 1) * TC)
                        ups = psA.tile([DI, TC], F32, name="ups", tag="mm")
                        nc.tensor.matmul(ups, lhsT=w["cwu0"],
                                         rhs=feat2x[:, c * TC:c * TC + TC],
                                         start=True, stop=False)
                        nc.tensor.matmul(ups, lhsT=w["cwu1"],
                                         rhs=feat2x[:, c * TC + 2:c * TC + 2 + TC],
                                         start=False, stop=True)
                        nc.scalar.activation(out=u_t[:, cs], in_=ups, func=AF.Silu,
                                             bias=w["conv_b"][:, :])
                        zps = psA.tile([DI, TC], F32, name="zps", tag="mm")
                        nc.tensor.matmul(zps, lhsT=w["inw_zT"],
                                         rhs=feat2x[0:DM, 3 + c * TC:3 + (c + 1) * TC],
                                         start=True, stop=True)
                        nc.scalar.activation(out=zs_t[:, cs], in_=zps, func=AF.Silu)
                    # x_proj / dt pass: exps batched per half, then one ln
                    for half in range(2):
                        spe = enw.tile([DI, HALF], F32, name="spe", tag="en")
                        for cc in range(NCH // 2):
                            c = half * (NCH // 2) + cc
                            cs = slice(c * TC, (c + 1) * TC)
                            xps = psA.tile([DR + 2 * DS, TC], F32, name="xps", tag="mm")
                            nc.tensor.matmul(xps, lhsT=w["xpwT"], rhs=u_t[:, cs],
                                             start=True, stop=True)
                            # x_proj rows host-permuted to [B(16), C(16), dt(4)]
                            bcc = small.tile([2 * DS, TC], BF16, name="bcc", tag="bcc")
                            nc.scalar.activation(out=bcc, in_=xps[0:2 * DS, :],
                                                 func=AF.Copy)
                            nc.scalar.dma_start(out=bc_dram[:, cs], in_=bcc)
                            dtr = small.tile([DR, TC], BF16, name="dtr", tag="dtr")
                            nc.scalar.activation(out=dtr,
                                                 in_=xps[2 * DS:2 * DS + DR, :],
                                                 func=AF.Copy)
                            dtps = psA.tile([DI, TC], F32, name="dtps", tag="mm")
                            nc.tensor.matmul(dtps, lhsT=w["dtpwT"], rhs=dtr,
                                             start=True, stop=True)
                            # softplus(x) = ln(1 + exp(x))
                            nc.scalar.activation(out=spe[:, cc * TC:(cc + 1) * TC],
                                                 in_=dtps, func=AF.Exp,
                                                 bias=w["dtp_b"][:, :])
                        nc.scalar.activation(out=dt_t[:, half * HALF:(half + 1) * HALF],
                                             in_=spe, func=AF.Ln, bias=one128[:, :])
                    nc.gpsimd.tensor_tensor(out=dtu_t, in0=dt_t, in1=u_t, op=OP.mult)

                with nc.named_scope(f"s{s}m{m}_scan"):
                    for q in range(2):
                        hs = q * HALF
                        qsl = slice(hs, hs + HALF)
                        yps = [psY.tile([DI, TC], F32, name=f"yps{k}", tag=f"yps{k}")
                               for k in range(HALF // TC)]
                        for n in range(DS):
                            en = enw.tile([DI, HALF], F32, name="en", tag="en")
                            nc.scalar.activation(out=en, in_=dt_t[:, qsl],
                                                 func=AF.Exp,
                                                 scale=w["A"][:, n:n + 1])
                            # B row n and C row n+16 in one broadcast DMA:
                            # bc[:, :HALF] = B_n, bc[:, HALF:] = C_n
                            bc_t = bcw.tile([DI, 2 * HALF], BF16, name="bc_t",
                                            tag="bc_t")
                            src = bass.AP(tensor=bc_dram.tensor,
                                          offset=bc_dram.offset + n * L + hs,
                                          ap=[[0, DI], [DS * L, 2], [1, HALF]])
                            nc.scalar.dma_start(out=bc_t, in_=src)
                            dbu = nwork.tile([DI, HALF], BF16, name="dbu", tag="dbu")
                            nc.gpsimd.tensor_tensor(out=dbu, in0=dtu_t[:, qsl],
                                                    in1=bc_t[:, 0:HALF], op=OP.mult)
                            h_t = nwork.tile([DI, HALF], BF16, name="h_t", tag="h_t")
                            init = 0.0 if q == 0 else carry[:, n:n + 1]
                            nc.vector.tensor_tensor_scan(
                                out=h_t, data0=en, data1=dbu,
                                initial=init, op0=OP.mult, op1=OP.add)
                            if q == 0:
                                nc.vector.tensor_copy(out=carry[:, n:n + 1],
                                                      in_=h_t[:, HALF - 1:HALF])
                            hc = nwork.tile([DI, HALF], BF16, name="hc", tag="hc")
                            heng = nc.vector if n in VHC else nc.gpsimd
                            heng.tensor_tensor(out=hc, in0=h_t,
                                               in1=bc_t[:, HALF:2 * HALF], op=OP.mult)
                            for k in range(HALF // TC):
                                nc.tensor.matmul(yps[k], lhsT=ident,
                                                 rhs=hc[:, k * TC:(k + 1) * TC],
                                                 start=(n == 0), stop=(n == DS - 1))
                        for k in range(HALF // TC):
                            cs = slice(hs + k * TC, hs + (k + 1) * TC)
                            tmp = small.tile([DI, TC], F32, name="ytmp", tag="ytmp")
                            nc.vector.scalar_tensor_tensor(
                                out=tmp, in0=u_t[:, cs], scalar=w["D"][:, :],
                                in1=yps[k], op0=OP.mult, op1=OP.add)
                            nc.gpsimd.tensor_tensor(out=yo_bf[:, cs], in0=tmp,
                                                    in1=zs_t[:, cs], op=OP.mult)

                with nc.named_scope(f"s{s}m{m}_post"):
                    # out_proj + per-position layernorm over the 64 channels.
                    # Stays inside the sqrt ACT table set (sqrt/square/copy/
                    # identity) -> no table thrash.
                    mps8 = psY.tile([NCH, TC], F32, name="mps8", tag="yps0")
                    msp8 = psY.tile([NCH, TC], F32, name="msp8", tag="yps1")
                    for c in range(NCH):
                        cs = slice(c * TC, (c + 1) * TC)
                        fps = psA.tile([DM, TC], F32, name="fps", tag="mm")
                        nc.tensor.matmul(fps, lhsT=w["opwT"], rhs=yo_bf[:, cs],
                                         start=True, stop=True)
                        nc.scalar.activation(out=fch_bf[:, cs], in_=fps, func=AF.Copy)
                        sq = small.tile([DM, TC], BF16, name="sq", tag="sq")
                        nc.scalar.activation(out=sq, in_=fch_bf[:, cs], func=AF.Square)
                        # accumulate chunk-c mean into row c of mps8 via a
                        # column-shifted 1/64 selector (other rows get 0)
                        wsl = wsel[:, c * NCH:(c + 1) * NCH]
                        nc.tensor.matmul(mps8, lhsT=wsl, rhs=fch_bf[:, cs],
                                         start=(c == 0), stop=(c == NCH - 1))
                        nc.tensor.matmul(msp8, lhsT=wsl, rhs=sq,
                                         start=(c == 0), stop=(c == NCH - 1))
                    mu8 = stp.tile([NCH, TC], F32, name="mu8", tag="mu8")
                    nc.scalar.activation(out=mu8, in_=mps8, func=AF.Copy)
                    mu2_8 = stp.tile([NCH, TC], F32, name="mu2_8", tag="mu2_8")
                    nc.vector.tensor_tensor(out=mu2_8, in0=mu8, in1=mu8, op=OP.mult)
                    var8 = stp.tile([NCH, TC], F32, name="var8", tag="var8")
                    nc.vector.tensor_tensor(out=var8, in0=msp8, in1=mu2_8,
                                            op=OP.subtract)
                    sd8 = stp.tile([NCH, TC], F32, name="sd8", tag="sd8")
                    nc.scalar.activation(out=sd8, in_=var8, func=AF.Sqrt,
                                         bias=eps8[:, :])
                    rstd8 = stp.tile([NCH, TC], F32, name="rstd8", tag="rstd8")
                    nc.vector.reciprocal(out=rstd8, in_=sd8)
                    for c in range(NCH):
                        cs = slice(c * TC, (c + 1) * TC)
                        # select chunk-c row of mu8/rstd8, broadcast to 64 parts
                        mbc = psA.tile([DM, TC], F32, name="mbc", tag="mm")
                        nc.tensor.matmul(mbc, lhsT=oh8[:, c * DM:(c + 1) * DM],
                                         rhs=mu8, start=True, stop=True)
                        rbc = psA.tile([DM, TC], F32, name="rbc", tag="mm")
                        nc.tensor.matmul(rbc, lhsT=oh8[:, c * DM:(c + 1) * DM],
                                         rhs=rstd8, start=True, stop=True)
                        t1 = small.tile([DM, TC], BF16, name="t1", tag="t1")
                        nc.vector.tensor_tensor(out=t1, in0=fch_bf[:, cs], in1=mbc,
                                                op=OP.subtract)
                        t2 = small.tile([DM, TC], BF16, name="t2", tag="t2")
                        nc.vector.tensor_tensor(out=t2, in0=t1, in1=rbc, op=OP.mult)
                        nc.scalar.activation(
                            out=feat2x[0:DM, 3 + c * TC:3 + (c + 1) * TC],
                            in_=t2, func=AF.Identity,
                            scale=w["ln_g"][:, :], bias=w["ln_b"][:, :])
                        if m == 0:
                            nc.scalar.activation(
                                out=feat2x[DM:2 * DM, 2 + c * TC:2 + (c + 1) * TC],
                                in_=t2, func=AF.Identity,
                                scale=w["ln_g"][:, :], bias=w["ln_b"][:, :])
                        else:
                            dps = psA.tile([C, TC], F32, name="dps", tag="mm")
                            nc.tensor.matmul(
                                dps, lhsT=sb_headT,
                                rhs=feat2x[0:DM, 3 + c * TC:3 + (c + 1) * TC],
                                start=True, stop=True)
                            nd = small.tile([C, TC], F32, name="nd", tag="nd")
                            nc.scalar.activation(out=nd, in_=dps, func=AF.Identity,
                                                 scale=-1.0, bias=sb_nhb[:, :])
                            zch2 = small.tile([C, TC], F32, name="zch2", tag="zch")
                            nc.sync.dma_start(out=zch2, in_=zc[s][:, cs])
                            oc = small.tile([C, TC], F32, name="oc", tag="ytmp")
                            nc.vector.tensor_tensor(out=oc, in0=zch2, in1=nd,
                                                    op=OP.add)
                            nc.gpsimd.dma_start(out=out[s][:, cs], in_=oc)

    nc.finalize()
    return nc


def _prep_maps(inputs):
    import ml_dtypes
    bf = ml_dtypes.bfloat16
    f = np.float32
    z = np.asarray(inputs["z_damaged"], dtype=f).reshape(B, C, L)

    base = {
        "ident": np.eye(128, dtype=bf),
        "emb_wT": np.ascontiguousarray(np.asarray(inputs["emb_w"], f).T),
        "emb_b": np.asarray(inputs["emb_b"], f).reshape(DM, 1),
        "head_wT": np.ascontiguousarray(np.asarray(inputs["head_w"], f).T).astype(bf),
        "neg_head_b": (-np.asarray(inputs["head_b"], f)).reshape(C, 1),
        "onehot8": np.concatenate(
            [np.eye(NCH, dtype=f)[:, c:c + 1] * np.ones((1, DM), f)
             for c in range(NCH)], axis=1),
        "wsel": np.concatenate(
            [np.eye(NCH, dtype=f)[c:c + 1, :] * np.full((DM, 1), 1.0 / DM, f)
             for c in range(NCH)], axis=1).astype(bf),
    }
    for m in (1, 2):
        p = f"m{m}_"
        inw = np.asarray(inputs[p + "in_proj_w"], f)  # [2DI, DM]
        w_u = inw[:DI]  # [DI, DM]
        cw = np.asarray(inputs[p + "conv_w"], f).reshape(DI, DK)
        # lhsT rows (k,m) -> cols d: w[d,k]*W_u[d,m]
        base[p + "cwu0"] = np.ascontiguousarray(np.concatenate(
            [cw[:, 0][None, :] * w_u.T, cw[:, 1][None, :] * w_u.T], axis=0)).astype(bf)
        base[p + "cwu1"] = np.ascontiguousarray(np.concatenate(
            [cw[:, 2][None, :] * w_u.T, cw[:, 3][None, :] * w_u.T], axis=0)).astype(bf)
        base[p + "inw_zT"] = np.ascontiguousarray(inw[DI:].T).astype(bf)
        base[p + "conv_b"] = np.asarray(inputs[p + "conv_b"], f).reshape(DI, 1)
        xpw = np.asarray(inputs[p + "x_proj_w"], f)  # rows: dt(4), B(16), C(16)
        xpw = np.concatenate([xpw[DR:], xpw[:DR]], axis=0)  # -> B, C, dt
        base[p + "xpwT"] = np.ascontiguousarray(xpw.T).astype(bf)
        base[p + "dtpwT"] = np.ascontiguousarray(
            np.asarray(inputs[p + "dt_proj_w"], f).T).astype(bf)
        base[p + "dtp_b"] = np.asarray(inputs[p + "dt_proj_b"], f).reshape(DI, 1)
        base[p + "A"] = -np.exp(np.asarray(inputs[p + "A_log"], f))
        base[p + "D"] = np.asarray(inputs[p + "D"], f).reshape(DI, 1)
        base[p + "opwT"] = np.ascontiguousarray(
            np.asarray(inputs[p + "out_proj_w"], f).T).astype(bf)
        base[p + "ln_g"] = np.asarray(inputs[f"ln{m}_g"], f).reshape(DM, 1)
        base[p + "ln_b"] = np.asarray(inputs[f"ln{m}_b"], f).reshape(DM, 1)

    maps = []
    for k in range(NCORES):
        mkp = dict(base)
        mkp["zc"] = np.ascontiguousarray(z[k * BPC:(k + 1) * BPC])
        maps.append(mkp)
    return maps


def _run(inputs, trace=False):
    from concourse.bass_utils import run_bass_kernel_spmd
    if "nc" not in _CACHE:
        _CACHE["nc"] = _build_program()
    nc = _CACHE["nc"]
    maps = _prep_maps(inputs)
    res = run_bass_kernel_spmd(nc, maps, core_ids=list(range(NCORES)), trace=trace)
    outs = [r["out"] for r in res.results]
    full = np.concatenate(outs, axis=0).reshape(B, C, H, W)
    return full, res


def kernel(**inputs):
    full, _ = _run(inputs, trace=False)
    return full
